# revision 1
# baseline (speedup 1.0000x reference)
"""Trainium2 Bass kernel for nn_Encoder_ATTENTION (gnn_message_passing).

Math (per (b, n)):
  wn     = normalize(w_r_weight[rid[b,n]])            (table prep, host)
  d      = <e[b,n,:], wn>
  e_tr   = e - d * wn                                  (unmasked; mask folded into coeffs)
  h      = tanh(W @ [z_q[b]; e_tr] + bias)             (z-part via per-b zw table, on-chip)
  alpha  = u_a . h + u_a_b
  E      = exp(alpha) * (rid < CNT_E)
  attn   = E / sum_n(E) + rw                           (softmax w/o max-sub; logits are small)
  out[b] = sum_n (attn * mask) * e_tr

Sharding: data-parallel over batch, 512 batch rows per core x 8 cores.
Host does layout-only prep (transposes/padding of weights + index tensors);
all batch-dependent math runs on device.

dtype scheme: PE matmul operands with wide moving dims are produced as
float32r (full-rate matmul); tiny matmuls (softmax sums) run plain fp32.
"""

import sys

import numpy as np


def _ensure_path():
    for p in ("/opt/trn_rl_repo", "/root/.axon_site/_ro/trn_rl_repo"):
        if p not in sys.path:
            sys.path.append(p)


_ensure_path()

from contextlib import ExitStack

import concourse.bacc as bacc
import concourse.bass as bass
import concourse.tile as tile
from concourse import mybir
from concourse.bass import IndirectOffsetOnAxis
from concourse.masks import make_identity

B, NB, DIM = 4096, 64, 256
H = 2 * DIM
NCORES = 8
BC = B // NCORES            # 512 batch rows per core
ROWS = BC * NB              # 32768 (b, n) rows per core
NT = ROWS // 128            # 256 tiles of 128 rows
TPB = 8                     # tiles per batch (softmax/output batching)
NBATCH = NT // TPB          # 32
CNT_E = 1000                # padding relation id
N_WR = CNT_E + 1            # 1001 rows in w_r table
N_ZQ = CNT_E                # 1000 rows in zq table
WN_PAD = 1024               # padded wn table rows

f32 = mybir.dt.float32
f32r = mybir.dt.float32r
i32 = mybir.dt.int32
AF = mybir.ActivationFunctionType
OP = mybir.AluOpType


def build_nc(nbatch=NBATCH):
    nc = bacc.Bacc("TRN2")

    e_d = nc.dram_tensor("e", [ROWS, DIM], f32, kind="ExternalInput")
    ridT_d = nc.dram_tensor("ridT", [128, NT], i32, kind="ExternalInput")
    rwT_d = nc.dram_tensor("rwT", [128, NT], f32, kind="ExternalInput")
    qoff_d = nc.dram_tensor("qoff", [128, BC // 128], i32, kind="ExternalInput")
    wn_d = nc.dram_tensor("wn", [WN_PAD, DIM], f32, kind="ExternalInput")
    zq_d = nc.dram_tensor("zq", [N_ZQ, DIM], f32r, kind="ExternalInput")
    WT_d = nc.dram_tensor("WT", [128, 4, H], f32r, kind="ExternalInput")
    bias_d = nc.dram_tensor("bias", [1, H], f32, kind="ExternalInput")
    ua_d = nc.dram_tensor("ua", [1, H], f32, kind="ExternalInput")
    uab_d = nc.dram_tensor("uab", [1, 1], f32, kind="ExternalInput")
    out_d = nc.dram_tensor("out", [BC, DIM], f32, kind="ExternalOutput")

    with tile.TileContext(nc) as tc, ExitStack() as ctx:
        const = ctx.enter_context(tc.tile_pool(name="const", bufs=1))
        epool = ctx.enter_context(tc.tile_pool(name="epool", bufs=2))
        gpool = ctx.enter_context(tc.tile_pool(name="gpool", bufs=4))
        wpool = ctx.enter_context(tc.tile_pool(name="wpool", bufs=3))
        etrp = ctx.enter_context(tc.tile_pool(name="etrp", bufs=12))
        tpool = ctx.enter_context(tc.tile_pool(name="tpool", bufs=3))
        hpool = ctx.enter_context(tc.tile_pool(name="hpool", bufs=3))
        scp = ctx.enter_context(tc.tile_pool(name="scp", bufs=2))
        abp = ctx.enter_context(tc.tile_pool(name="abp", bufs=3))
        czp = ctx.enter_context(tc.tile_pool(name="czp", bufs=2))
        osp = ctx.enter_context(tc.tile_pool(name="osp", bufs=2))
        stp = ctx.enter_context(tc.tile_pool(name="stp", bufs=3))
        rsp = ctx.enter_context(tc.tile_pool(name="rsp", bufs=2))

        tpps = ctx.enter_context(tc.tile_pool(name="tpps", bufs=2, space="PSUM"))
        hps = ctx.enter_context(tc.tile_pool(name="hps", bufs=2, space="PSUM"))
        ops_ = ctx.enter_context(tc.tile_pool(name="ops", bufs=2, space="PSUM"))
        sps = ctx.enter_context(tc.tile_pool(name="sps", bufs=1, space="PSUM"))
        rbcp = ctx.enter_context(tc.tile_pool(name="rbcp", bufs=1, space="PSUM"))

        # ---------- constants ----------
        ident_f = const.tile([128, 128], f32)
        make_identity(nc, ident_f[:])
        ident = const.tile([128, 128], f32r)
        nc.vector.tensor_copy(ident[:], ident_f[:])
        zeros128 = const.tile([128, TPB * 16], f32)
        nc.gpsimd.memset(zeros128[:], 0.0)

        # blkpat[p, g] = 1.0 if p // 64 == g else 0.0          [128, 2]
        io2 = const.tile([128, 2], i32)
        nc.gpsimd.iota(io2[:], pattern=[[-64, 2]], base=0, channel_multiplier=1)
        bp0 = const.tile([128, 2], f32)
        bp1 = const.tile([128, 2], f32)
        nc.vector.tensor_scalar(out=bp0[:], in0=io2[:], scalar1=0, scalar2=None, op0=OP.is_ge)
        nc.vector.tensor_scalar(out=bp1[:], in0=io2[:], scalar1=63, scalar2=None, op0=OP.is_le)
        blkpat = const.tile([128, 2], f32)
        nc.vector.tensor_tensor(out=blkpat[:], in0=bp0[:], in1=bp1[:], op=OP.mult)

        # O2T[g, c] = 1.0 if c // 64 == g else 0.0             [2, 128]
        io3 = const.tile([2, 128], i32)
        nc.gpsimd.iota(io3[:], pattern=[[1, 128]], base=0, channel_multiplier=-64)
        ot0 = const.tile([2, 128], f32)
        ot1 = const.tile([2, 128], f32)
        nc.vector.tensor_scalar(out=ot0[:], in0=io3[:], scalar1=0, scalar2=None, op0=OP.is_ge)
        nc.vector.tensor_scalar(out=ot1[:], in0=io3[:], scalar1=63, scalar2=None, op0=OP.is_le)
        O2T = const.tile([2, 128], f32r)
        nc.vector.tensor_tensor(out=O2T[:], in0=ot0[:], in1=ot1[:], op=OP.mult)

        # ---------- broadcast / table loads ----------
        # (partition-step-0 DMA broadcast crashes the exec unit on this
        # runtime; broadcast across partitions via a PE outer product instead)
        ones1 = const.tile([1, 128], f32)
        nc.gpsimd.memset(ones1[:], 1.0)
        ua_row = const.tile([1, H], f32)
        nc.sync.dma_start(out=ua_row[:], in_=ua_d[:])
        bias_row = const.tile([1, H], f32)
        nc.sync.dma_start(out=bias_row[:], in_=bias_d[:])
        uab_row = const.tile([1, 1], f32)
        nc.sync.dma_start(out=uab_row[:], in_=uab_d[:])

        bc_ps = hps.tile([128, H], f32, tag="hps")
        nc.tensor.matmul(out=bc_ps[:], lhsT=ones1[:], rhs=ua_row[:])
        u_ab = const.tile([128, H], f32)
        nc.scalar.copy(u_ab[:], bc_ps[:])
        bc_ps2 = hps.tile([128, H], f32, tag="hps")
        nc.tensor.matmul(out=bc_ps2[:], lhsT=ones1[:], rhs=bias_row[:])
        biasb = const.tile([128, H], f32)
        nc.scalar.copy(biasb[:], bc_ps2[:])
        bc_ps3 = tpps.tile([128, DIM], f32, tag="tp")
        nc.tensor.matmul(out=bc_ps3[:, 0:1], lhsT=ones1[:], rhs=uab_row[:])
        uab_b = const.tile([128, 1], f32)
        nc.scalar.copy(uab_b[:], bc_ps3[:, 0:1])
        WTs = const.tile([128, 4, H], f32r)
        nc.sync.dma_start(out=WTs[:], in_=WT_d[:])
        ridTs = const.tile([128, NT], i32)
        nc.sync.dma_start(out=ridTs[:], in_=ridT_d[:])
        rwTs = const.tile([128, NT], f32)
        nc.sync.dma_start(out=rwTs[:], in_=rwT_d[:])
        qoffs = const.tile([128, BC // 128], i32)
        nc.sync.dma_start(out=qoffs[:], in_=qoff_d[:])

        # mask / masked rw, in tile-major layout [128, NT]
        ridTf = const.tile([128, NT], f32)
        nc.vector.tensor_copy(ridTf[:], ridTs[:])
        maskT = const.tile([128, NT], f32)
        nc.vector.tensor_scalar(out=maskT[:], in0=ridTf[:], scalar1=float(CNT_E), scalar2=None, op0=OP.is_lt)
        rwmT = const.tile([128, NT], f32)
        nc.vector.tensor_tensor(out=rwmT[:], in0=rwTs[:], in1=maskT[:], op=OP.mult)

        # ---------- zw table: zw[b] = W_z @ zq[q_rid[b]] + bias   [128, 4, H] ----------
        z_all = const.tile([128, BC // 128, DIM], f32r)
        for j in range(BC // 128):
            nc.gpsimd.indirect_dma_start(
                out=z_all[:, j, :],
                out_offset=None,
                in_=zq_d[:],
                in_offset=IndirectOffsetOnAxis(ap=qoffs[:, j : j + 1], axis=0),
            )
        zw_all = const.tile([128, BC // 128, H], f32r)
        for j in range(BC // 128):
            tp = tpps.tile([128, DIM], f32r, tag="tp")
            for k in range(2):
                nc.tensor.transpose(
                    out=tp[:, 128 * k : 128 * (k + 1)],
                    in_=z_all[:, j, 128 * k : 128 * (k + 1)],
                    identity=ident[:],
                )
            zT = tpool.tile([128, DIM], f32r, tag="eT")
            nc.scalar.copy(zT[:], tp[:])
            zw_ps = hps.tile([128, H], f32, tag="hps")
            for k in range(2):
                nc.tensor.matmul(
                    out=zw_ps[:],
                    lhsT=zT[:, 128 * k : 128 * (k + 1)],
                    rhs=WTs[:, k, :],
                    start=(k == 0),
                    stop=(k == 1),
                    skip_group_check=True,
                )
            nc.vector.tensor_tensor(out=zw_all[:, j, :], in0=zw_ps[:], in1=biasb[:], op=OP.add)

        # ---------- main loop ----------
        e_re = e_d[:].rearrange("(t p) d -> p t d", p=128)  # [128, NT, DIM]

        for bt in range(nbatch):
            t0 = bt * TPB
            e8 = epool.tile([128, TPB, DIM], f32, tag="e8")
            nc.sync.dma_start(out=e8[:], in_=e_re[:, t0 : t0 + TPB, :])

            alpha_b = abp.tile([128, TPB], f32, tag="alpha")
            etrs = []
            for s in range(TPB):
                t = t0 + s
                G = gpool.tile([128, DIM], f32, tag="G")
                nc.gpsimd.indirect_dma_start(
                    out=G[:],
                    out_offset=None,
                    in_=wn_d[:],
                    in_offset=IndirectOffsetOnAxis(ap=ridTs[:, t : t + 1], axis=0),
                )
                et = e8[:, s, :]
                X = wpool.tile([128, DIM], f32, tag="X")
                dv = wpool.tile([128, 1], f32, tag="dv")
                nc.vector.tensor_tensor(out=X[:], in0=et, in1=G[:], op=OP.mult)
                nc.vector.tensor_reduce(out=dv[:], in_=X[:], axis=mybir.AxisListType.X, op=OP.add)
                dG = wpool.tile([128, DIM], f32, tag="dG")
                nc.vector.tensor_scalar(out=dG[:], in0=G[:], scalar1=dv[:], scalar2=None, op0=OP.mult)
                etr = etrp.tile([128, DIM], f32r, tag="etr")
                nc.vector.tensor_tensor(out=etr[:], in0=et, in1=dG[:], op=OP.subtract)
                etrs.append(etr)

                tp = tpps.tile([128, DIM], f32r, tag="tp")
                for k in range(2):
                    nc.tensor.transpose(
                        out=tp[:, 128 * k : 128 * (k + 1)],
                        in_=etr[:, 128 * k : 128 * (k + 1)],
                        identity=ident[:],
                    )
                eT = tpool.tile([128, DIM], f32r, tag="eT")
                nc.scalar.copy(eT[:], tp[:])

                h_ps = hps.tile([128, H], f32, tag="hps")
                nc.tensor.matmul(
                    out=h_ps[:], lhsT=eT[:, 0:128], rhs=WTs[:, 2, :],
                    start=True, stop=False, skip_group_check=True,
                )
                nc.tensor.matmul(
                    out=h_ps[:], lhsT=eT[:, 128:256], rhs=WTs[:, 3, :],
                    start=False, stop=False, skip_group_check=True,
                )
                b0 = 2 * t
                stage = stp.tile([2, H], f32r, tag="stage")
                nc.sync.dma_start(out=stage[:], in_=zw_all[b0 % 128 : b0 % 128 + 2, b0 // 128, :])
                nc.tensor.matmul(
                    out=h_ps[:], lhsT=O2T[:], rhs=stage[:],
                    start=False, stop=True, skip_group_check=True,
                )

                h = hpool.tile([128, H], f32, tag="h")
                nc.scalar.activation(out=h[:], in_=h_ps[:], func=AF.Tanh)
                sc = scp.tile([128, H], f32, tag="sc")
                nc.vector.tensor_tensor(out=sc[:], in0=h[:], in1=u_ab[:], op=OP.mult)
                nc.vector.tensor_reduce(
                    out=alpha_b[:, s : s + 1], in_=sc[:], axis=mybir.AxisListType.X, op=OP.add
                )

            # ----- batch tail: softmax + coeffs + output reduction -----
            Eb = abp.tile([128, TPB], f32, tag="Eb")
            nc.scalar.activation(out=Eb[:], in_=alpha_b[:], func=AF.Exp, bias=uab_b[:, 0:1])
            Em = abp.tile([128, TPB], f32, tag="Em")
            nc.vector.tensor_tensor(out=Em[:], in0=Eb[:], in1=maskT[:, t0 : t0 + TPB], op=OP.mult)

            s_ps = sps.tile([2, TPB], f32, tag="sps")
            nc.tensor.matmul(out=s_ps[:], lhsT=blkpat[:], rhs=Em[:])
            rS = rsp.tile([2, TPB], f32, tag="rS")
            nc.vector.reciprocal(rS[:], s_ps[:])
            rS_r = rsp.tile([2, TPB], f32r, tag="rSr")
            nc.vector.tensor_copy(rS_r[:], rS[:])
            rbc_ps = rbcp.tile([128, TPB], f32, tag="rbc")
            nc.tensor.matmul(out=rbc_ps[:], lhsT=O2T[:], rhs=rS_r[:])

            coeff = abp.tile([128, TPB], f32, tag="coeff")
            nc.vector.tensor_tensor(out=coeff[:], in0=Em[:], in1=rbc_ps[:], op=OP.mult)
            nc.vector.tensor_tensor(out=coeff[:], in0=coeff[:], in1=rwmT[:, t0 : t0 + TPB], op=OP.add)

            # Cz: [128, TPB*16]; block s has coeff at cols (2s, 2s+1), zeros elsewhere
            cz = czp.tile([128, TPB * 16], f32r, tag="cz")
            nc.scalar.copy(cz[:], zeros128[:])
            for s in range(TPB):
                nc.vector.tensor_scalar(
                    out=cz[:, 16 * s + 2 * s : 16 * s + 2 * s + 2],
                    in0=blkpat[:],
                    scalar1=coeff[:, s : s + 1],
                    scalar2=None,
                    op0=OP.mult,
                )

            o_ps = ops_.tile([2 * TPB, DIM], f32, tag="ops")
            for s in range(TPB):
                nc.tensor.matmul(
                    out=o_ps[:],
                    lhsT=cz[:, 16 * s : 16 * (s + 1)],
                    rhs=etrs[s][:],
                    start=(s == 0),
                    stop=(s == TPB - 1),
                    skip_group_check=True,
                )
            outS = osp.tile([2 * TPB, DIM], f32, tag="outS")
            nc.scalar.copy(outS[:], o_ps[:])
            nc.sync.dma_start(out=out_d[2 * TPB * bt : 2 * TPB * (bt + 1), :], in_=outS[:])

    nc.finalize()
    return nc


_NC = None


def _get_nc():
    global _NC
    if _NC is None:
        _NC = build_nc()
    return _NC


def _prep_in_maps(inputs):
    e = np.ascontiguousarray(np.asarray(inputs["batch_nei_e_emb"], dtype=np.float32))
    rid = np.asarray(inputs["batch_nei_rid"]).astype(np.int32)
    rw = np.asarray(inputs["batch_nei_rw"], dtype=np.float32)
    qr = np.asarray(inputs["batch_q_rid"]).astype(np.int32)

    w = np.asarray(inputs["w_r_weight"], dtype=np.float32)
    nrm = np.maximum(np.linalg.norm(w, axis=1, keepdims=True), 1e-12)
    wn = np.zeros((WN_PAD, DIM), np.float32)
    wn[:N_WR] = w / nrm
    WT = np.asarray(inputs["attn_W_w"], dtype=np.float32).T  # [in=512, out=512]
    WT4 = np.ascontiguousarray(WT.reshape(4, 128, H).transpose(1, 0, 2))  # [128, 4, H]
    zq = np.ascontiguousarray(np.asarray(inputs["zq_weight"], dtype=np.float32))
    bias = np.asarray(inputs["attn_W_b"], dtype=np.float32).reshape(1, H)
    ua = np.asarray(inputs["u_a_w"], dtype=np.float32).reshape(1, H)
    uab = np.asarray(inputs["u_a_b"], dtype=np.float32).reshape(1, 1)

    in_maps = []
    for c in range(NCORES):
        sl = slice(BC * c, BC * (c + 1))
        ec = np.ascontiguousarray(e[sl].reshape(ROWS, DIM))
        ridc = rid[sl].reshape(ROWS)
        rwc = rw[sl].reshape(ROWS)
        qc = qr[sl]
        in_maps.append(
            {
                "e": ec,
                "ridT": np.ascontiguousarray(ridc.reshape(NT, 128).T),
                "rwT": np.ascontiguousarray(rwc.reshape(NT, 128).T),
                "qoff": np.ascontiguousarray(qc.reshape(BC // 128, 128).T),
                "wn": wn,
                "zq": zq,
                "WT": WT4,
                "bias": bias,
                "ua": ua,
                "uab": uab,
            }
        )
    return in_maps


def run_cores(inputs, trace=False, tmpdir=None):
    from concourse.bass_utils import run_bass_kernel_spmd

    nc = _get_nc()
    in_maps = _prep_in_maps(inputs)
    res = run_bass_kernel_spmd(
        nc, in_maps, core_ids=list(range(NCORES)), trace=trace, tmpdir=tmpdir
    )
    out = np.concatenate([res.results[c]["out"] for c in range(NCORES)], axis=0)
    return out, res


def kernel(**inputs):
    out, _ = run_cores(inputs, trace=False)
    return out

